# revision 6
# baseline (speedup 1.0000x reference)
"""Varlen causal sliding-window attention with per-head sink logits, on 8 trn2 cores.

Wall time under the axon tunnel is transfer-bound (~20 ms/MiB each way), so the
wire format is int8 with per-(token,head) scales for q/k/v and int8 with
per-output-channel scales for O; all dequant/requant runs on-device where
compute is ~free (cost-model exec ~0.3 ms vs seconds of transfer).

Sharding: data-parallel over (batch, head-group). Each core gets one batch's
tokens and 16/PB contiguous q-heads (PB = 8//B parts per batch) plus the
matching kv-heads.

Device inputs (per core):
  q [S, HL*128] int8, qs [128, HL*NT] f32   (qs[p, h*NT+t] = scale of token
  t*128+p, head h); k/v [S, KVL*128] int8 with sks/vs [128, KVL*NT] f32
  (sks premultiplied by softmax SCALE); sinks [128, HL] f32.
Device outputs:
  o [HL*128, S] int8 (O^T layout), om [HL*128, 1] f32 (per-channel absmax;
  host dequant is o * om/127).

Device kernel per head:
  - load: int8 tiles DMA'd natural-layout; q dequants per-token via DVE
    tensor_scalar (int8 x [128,1] f32 -> f16) then PE-transposes per 128-block
    into qT; k converts int8->f16 (scales NOT applied) and PE-transposes into
    kT; v dequants in place (lhsT consumed natural).
  - QK^T per 128-key tile into PSUM; exp evicts PSUM->SBUF f16 probs with the
    per-key scale folded into the activation's per-partition scale AP
    (logit = qdeq . k_int, exp scale = SCALE*ks_key); triangular masks fix the
    two band edges.
  - PV + ones-matmul denominator per 256-query span accumulate O^T and D in
    one PSUM bank; DVE adds exp(sink), reciprocal, multiply into an f32 O^T
    row; after all spans: rowmax(|O|), quantize row to int8, DMA out.
"""

import sys

sys.path.insert(0, "/opt/trn_rl_repo")

import numpy as np

NUM_HEADS = 16
NUM_KV_HEADS = 4
HEAD_DIM = 128
WINDOW = 1024
SCALE = 0.08838834764831845
TILE = 128

_CACHE = {}
_JIT = {}


def _band_width(kj, S):
    # keys in tile kj are visible to queries q with 0 <= q - k <= WINDOW
    # -> q in [kj*TILE, kj*TILE + WINDOW + TILE), clipped to S
    return min(S, kj * TILE + WINDOW + TILE) - kj * TILE


def _chunks(w):
    # split [0, w) at 512 boundaries (PSUM bank) for matmul outputs
    out = []
    c0 = 0
    while c0 < w:
        out.append((c0, min(512, w - c0)))
        c0 += 512
    return out


def build_nc(S, HL, KVL):
    import concourse.bacc as bacc
    import concourse.mybir as mybir
    from concourse.masks import make_identity, make_lower_triangular, make_upper_triangular
    from concourse.tile import TileContext

    f32 = mybir.dt.float32
    f16 = mybir.dt.float16
    i8 = mybir.dt.int8
    NT = S // TILE
    WMAX = min(S, WINDOW + TILE)
    SUMW = sum(_band_width(kj, S) for kj in range(NT))
    OFF = np.cumsum([0] + [_band_width(kj, S) for kj in range(NT)]).tolist()
    SPAN = 256
    NSPAN = S // SPAN

    nc = bacc.Bacc()
    q_d = nc.dram_tensor("q", [S, HL * TILE], i8, kind="ExternalInput")
    # kv: per local kv head, 128 k-cols then 128 v-cols (interleaved per head)
    kv_d = nc.dram_tensor("kv", [S, KVL * 2 * TILE], i8, kind="ExternalInput")
    # aux columns: qs [HL*NT] | sks [KVL*NT] | vs [KVL*NT] | sinks [HL]
    aux_d = nc.dram_tensor(
        "aux", [TILE, HL * NT + 2 * KVL * NT + HL], f32, kind="ExternalInput"
    )
    # last 4 int8 cols of each row hold the f32 per-channel absmax (bitcast)
    o_d = nc.dram_tensor("o", [HL * TILE, S + 4], i8, kind="ExternalOutput")

    with TileContext(nc) as tc:
        with (
            tc.tile_pool(name="const", bufs=1) as const_pool,
            tc.tile_pool(name="qi8", bufs=2) as qi8_pool,
            tc.tile_pool(name="kvi8", bufs=2) as kvi8_pool,
            tc.tile_pool(name="dq", bufs=3) as dq_pool,
            tc.tile_pool(name="qT", bufs=3) as qT_pool,
            tc.tile_pool(name="kT", bufs=2) as kT_pool,
            tc.tile_pool(name="vv", bufs=2) as v_pool,
            tc.tile_pool(name="pT", bufs=3) as pT_pool,
            tc.tile_pool(name="dsb", bufs=3) as d_pool,
            tc.tile_pool(name="orow", bufs=2) as orow_pool,
            tc.tile_pool(name="oi8", bufs=2) as oi8_pool,
            tc.tile_pool(name="stat", bufs=4) as stat_pool,
            tc.tile_pool(name="spsum", bufs=1, space="PSUM") as s_psum,
            tc.tile_pool(name="opsum", bufs=2, space="PSUM") as o_psum,
            tc.tile_pool(name="tpsum", bufs=2, space="PSUM") as t_psum,
        ):
            mask_diag = const_pool.tile([TILE, TILE], f16)  # valid: q >= k
            mask_win = const_pool.tile([TILE, TILE], f16)  # valid: q <= k
            make_upper_triangular(nc, mask_diag[:], val=1.0, diag=True)
            make_lower_triangular(nc, mask_win[:], val=1.0, diag=True)
            ones = const_pool.tile([TILE, TILE], f16)
            nc.vector.memset(ones[:], 1.0)
            ident = const_pool.tile([TILE, TILE], f16)
            make_identity(nc, ident[:])
            AUXW = HL * NT + 2 * KVL * NT + HL
            aux_sb = const_pool.tile([TILE, AUXW], f32)
            nc.sync.dma_start(out=aux_sb[:], in_=aux_d[:, :])
            qs_sb = aux_sb[:, : HL * NT]
            sks_sb = aux_sb[:, HL * NT : HL * NT + KVL * NT]
            vs_sb = aux_sb[:, HL * NT + KVL * NT : HL * NT + 2 * KVL * NT]
            esk = const_pool.tile([TILE, HL], f32)
            nc.scalar.activation(
                esk[:],
                aux_sb[:, HL * NT + 2 * KVL * NT :],
                mybir.ActivationFunctionType.Exp,
            )

            kT_sb = None
            v_by_kv = {}
            pT_by_hl = {}

            def qk_phase(hl):
                nonlocal kT_sb
                kv = hl // 4 if HL >= 4 else 0
                if hl % 4 == 0 or kT_sb is None:
                    # ---- K: int8 natural -> f16 (unscaled) -> kT via PE ----
                    ki8 = kvi8_pool.tile([TILE, NT * TILE], i8, tag="ki8")
                    nc.sync.dma_start(
                        out=ki8[:].rearrange("p (t d) -> p t d", d=TILE),
                        in_=kv_d[:, kv * 2 * TILE : kv * 2 * TILE + TILE].rearrange(
                            "(t p) d -> p t d", p=TILE
                        ),
                    )
                    kT_sb = kT_pool.tile([TILE, S], f16, tag="kT")
                    for t in range(NT):
                        dq = dq_pool.tile([TILE, TILE], f16, tag="dq")
                        nc.vector.tensor_copy(dq[:], ki8[:, t * TILE : (t + 1) * TILE])
                        tp = t_psum.tile([TILE, TILE], f16, tag="tp")
                        nc.tensor.transpose(tp[:], dq[:], ident[:])
                        nc.vector.tensor_copy(kT_sb[:, t * TILE : (t + 1) * TILE], tp[:])
                    # ---- V: int8 natural, dequant in place ----
                    vi8 = kvi8_pool.tile([TILE, NT * TILE], i8, tag="vi8")
                    nc.gpsimd.dma_start(
                        out=vi8[:].rearrange("p (t d) -> p t d", d=TILE),
                        in_=kv_d[
                            :, kv * 2 * TILE + TILE : (kv + 1) * 2 * TILE
                        ].rearrange("(t p) d -> p t d", p=TILE),
                    )
                    v_sb = v_pool.tile([TILE, NT * TILE], f16, tag="vv")
                    for t in range(NT):
                        nc.vector.tensor_scalar_mul(
                            v_sb[:, t * TILE : (t + 1) * TILE],
                            vi8[:, t * TILE : (t + 1) * TILE],
                            aux_sb[:, HL * NT + KVL * NT + kv * NT + t : HL * NT + KVL * NT + kv * NT + t + 1],
                        )
                    v_by_kv[kv] = v_sb

                # ---- Q: int8 natural -> dequant (per-token scale) -> qT ----
                qi8 = qi8_pool.tile([TILE, NT * TILE], i8, tag="qi8")
                nc.sync.dma_start(
                    out=qi8[:].rearrange("p (t d) -> p t d", d=TILE),
                    in_=q_d[:, hl * TILE : (hl + 1) * TILE].rearrange(
                        "(t p) d -> p t d", p=TILE
                    ),
                )
                qT_sb = qT_pool.tile([TILE, S], f16, tag="qT")
                for t in range(NT):
                    dq = dq_pool.tile([TILE, TILE], f16, tag="dq")
                    nc.vector.tensor_scalar_mul(
                        dq[:],
                        qi8[:, t * TILE : (t + 1) * TILE],
                        aux_sb[:, hl * NT + t : hl * NT + t + 1],
                    )
                    tp = t_psum.tile([TILE, TILE], f16, tag="tp")
                    nc.tensor.transpose(tp[:], dq[:], ident[:])
                    nc.vector.tensor_copy(qT_sb[:, t * TILE : (t + 1) * TILE], tp[:])

                pT = pT_pool.tile([TILE, SUMW], f16, tag="pT")
                pT_by_hl[hl] = pT

                # ---- QK^T + exp (per-key scale via activation scale AP) ----
                for kj in range(NT):
                    w = _band_width(kj, S)
                    off = OFF[kj]
                    q0 = kj * TILE
                    s_ps = s_psum.tile([TILE, WMAX], f32, tag="s")
                    for c0, cw in _chunks(w):
                        nc.tensor.matmul(
                            s_ps[:, c0 : c0 + cw],
                            lhsT=kT_sb[:, kj * TILE : (kj + 1) * TILE],
                            rhs=qT_sb[:, q0 + c0 : q0 + c0 + cw],
                            start=True,
                            stop=True,
                        )
                    nc.scalar.activation(
                        pT[:, off : off + w],
                        s_ps[:, :w],
                        mybir.ActivationFunctionType.Exp,
                        scale=aux_sb[:, HL * NT + kv * NT + kj : HL * NT + kv * NT + kj + 1],
                    )
                    nc.vector.tensor_mul(
                        pT[:, off : off + TILE],
                        pT[:, off : off + TILE],
                        mask_diag[:],
                    )
                    if kj * TILE + WINDOW + TILE <= S:
                        nc.vector.tensor_mul(
                            pT[:, off + WINDOW : off + WINDOW + TILE],
                            pT[:, off + WINDOW : off + WINDOW + TILE],
                            mask_win[:],
                        )

            def pv_phase(hl):
                kv = hl // 4 if HL >= 4 else 0
                v_sb = v_by_kv[kv]
                pT = pT_by_hl.pop(hl)
                out_row = orow_pool.tile([TILE, S], f32, tag="orow")
                # ---- PV + denominator, per query span ----
                # od_ps: one PSUM bank; cols [0,SPAN) = O^T, [SPAN,2*SPAN) = D
                for sp in range(NSPAN):
                    lo, hi = sp * SPAN, (sp + 1) * SPAN
                    ktiles = []
                    for kj in range(NT):
                        w = _band_width(kj, S)
                        qlo = max(kj * TILE, lo)
                        qhi = min(kj * TILE + w, hi)
                        if qhi > qlo:
                            ktiles.append((kj, qlo, qhi))
                    # full-span writers first (uniform psum zero-region state)
                    ktiles.sort(key=lambda t: 0 if (t[1] == lo and t[2] == hi) else 1)
                    assert ktiles[0][1] == lo and ktiles[0][2] == hi, (S, sp)

                    od_ps = o_psum.tile([TILE, 2 * SPAN], f32, tag="od")
                    n = len(ktiles)
                    for i, (kj, qlo, qhi) in enumerate(ktiles):
                        rel_p = OFF[kj] + (qlo - kj * TILE)
                        rel_o = qlo - lo
                        ln = qhi - qlo
                        rhs = pT[:, rel_p : rel_p + ln]
                        nc.tensor.matmul(
                            od_ps[:, rel_o : rel_o + ln],
                            lhsT=v_sb[:, kj * TILE : (kj + 1) * TILE],
                            rhs=rhs,
                            start=(i == 0),
                            stop=False,
                        )
                        nc.tensor.matmul(
                            od_ps[:, SPAN + rel_o : SPAN + rel_o + ln],
                            lhsT=ones[:, :],
                            rhs=rhs,
                            start=False,
                            stop=(i == n - 1),
                        )

                    d_sb = d_pool.tile([TILE, SPAN], f32, tag="d_sb")
                    nc.vector.tensor_scalar_add(
                        d_sb[:], od_ps[:, SPAN : 2 * SPAN], esk[:, hl : hl + 1]
                    )
                    nc.vector.reciprocal(d_sb[:], d_sb[:])
                    nc.vector.tensor_mul(out_row[:, lo:hi], od_ps[:, :SPAN], d_sb[:])

                # ---- per-channel int8 quantization of the O^T row ----
                m = stat_pool.tile([TILE, 1], f32, tag="m")
                nc.vector.tensor_reduce(
                    out=m[:],
                    in_=out_row[:],
                    axis=mybir.AxisListType.X,
                    op=mybir.AluOpType.max,
                    apply_absolute_value=True,
                )
                nc.vector.tensor_scalar_max(m[:], m[:], 1e-20)
                r = stat_pool.tile([TILE, 1], f32, tag="r")
                nc.vector.reciprocal(r[:], m[:])
                nc.vector.tensor_scalar_mul(r[:], r[:], 127.0)
                oi8 = oi8_pool.tile([TILE, S], i8, tag="oi8")
                nc.vector.tensor_scalar_mul(oi8[:], out_row[:], r[:, 0:1])
                # out-DMA on SWDGE: keeps SP's FIFO free for the next
                # head's loads
                nc.gpsimd.dma_start(
                    out=o_d[hl * TILE : (hl + 1) * TILE, :S], in_=oi8[:]
                )
                nc.sync.dma_start(
                    out=o_d[hl * TILE : (hl + 1) * TILE, S : S + 4],
                    in_=m[:].bitcast(i8),
                )

            # software pipeline across heads: QK(hl+1) is emitted before
            # PV(hl) so PV never chases a just-issued exp
            qk_phase(0)
            for hl in range(1, HL):
                qk_phase(hl)
                pv_phase(hl - 1)
            pv_phase(HL - 1)
    # Bacc lowering (wait splitting, reg alloc) must run before serialization;
    # nothing on the PJRT path calls it for us.
    nc.finalize()
    return nc


def _get_nc(S, HL, KVL):
    key = (S, HL, KVL)
    if key not in _CACHE:
        _CACHE[key] = build_nc(S, HL, KVL)
    return _CACHE[key]


def _get_jits():
    if "prep" in _JIT:
        return _JIT
    import jax
    import jax.numpy as jnp

    _JIT["cpu"] = jax.local_devices(backend="cpu")[0]

    def quant(x, nh):
        T = x.shape[0]
        xr = x.reshape(T, nh, HEAD_DIM)
        s = jnp.maximum(jnp.max(jnp.abs(xr), axis=2) / 127.0, 1e-12)  # [T,nh]
        xi = jnp.clip(jnp.round(xr / s[:, :, None]), -127, 127).astype(jnp.int8)
        return xi.reshape(T, nh * HEAD_DIM), s

    def prep(q, k, v):
        qi, qs = quant(q, NUM_HEADS)
        ki, ks = quant(k, NUM_KV_HEADS)
        vi, vs = quant(v, NUM_KV_HEADS)
        T = k.shape[0]
        kvi = jnp.concatenate(
            [
                ki.reshape(T, NUM_KV_HEADS, HEAD_DIM),
                vi.reshape(T, NUM_KV_HEADS, HEAD_DIM),
            ],
            axis=2,
        ).reshape(T, NUM_KV_HEADS * 2 * HEAD_DIM)
        return qi, qs, kvi, ks, vs

    def assemble(o_list, B, S, PB, HL):
        raw = jnp.stack(o_list)  # [8, HL*128, S+4] int8
        oi8 = raw[:, :, :S]
        om = jax.lax.bitcast_convert_type(raw[:, :, S : S + 4], jnp.float32)
        om = om[:, :, None]  # [8, HL*128, 1] f32
        o = oi8.astype(jnp.float32) * (om / 127.0)  # [8, HL*128, S]
        o = o.reshape(B, PB, HL * HEAD_DIM, S)
        # out[b*S + s_, p*HL*128 + c] = o[b, p, c, s_]
        out = o.transpose(0, 3, 1, 2).reshape(B * S, NUM_HEADS * HEAD_DIM)
        return out

    _JIT["prep"] = jax.jit(prep)
    _JIT["assemble"] = jax.jit(assemble, static_argnums=(1, 2, 3, 4))
    return _JIT


def _scales_tile(s, scale_mul=1.0):
    """[S, nh] f32 per-core scales -> [128, nh*NT] device layout
    (column h*NT+t holds scales of tokens t*128 + p)."""
    S, nh = s.shape
    NT = S // TILE
    out = (s.reshape(NT, TILE, nh) * scale_mul).transpose(1, 2, 0)
    return np.ascontiguousarray(out.reshape(TILE, nh * NT), dtype=np.float32)


def kernel(q, k, v, sinks, batch, seqlen):
    import jax

    from concourse.bass_utils import run_bass_kernel_spmd

    q = np.asarray(q)
    k = np.asarray(k)
    v = np.asarray(v)
    sinks = np.asarray(sinks)
    B = int(batch)
    S = int(seqlen)
    assert 8 % B == 0, B
    PB = 8 // B  # head-parts per batch
    HL = NUM_HEADS // PB
    KVL = max(1, NUM_KV_HEADS // PB)

    nc = _get_nc(S, HL, KVL)
    jits = _get_jits()

    with jax.default_device(jits["cpu"]):
        qi, qs, kvi, ks, vs = [np.asarray(a) for a in jits["prep"](q, k, v)]

    in_maps = []
    shards = []
    for c in range(8):
        b, p = divmod(c, PB)
        tok = slice(b * S, (b + 1) * S)
        hsl = slice(p * HL * HEAD_DIM, (p + 1) * HL * HEAD_DIM)
        kv_lo = (p * HL) // 4
        kvsl = slice(kv_lo * 2 * HEAD_DIM, (kv_lo + KVL) * 2 * HEAD_DIM)
        aux = np.concatenate(
            [
                _scales_tile(qs[tok, p * HL : (p + 1) * HL]),
                _scales_tile(ks[tok, kv_lo : kv_lo + KVL], SCALE),
                _scales_tile(vs[tok, kv_lo : kv_lo + KVL]),
                np.broadcast_to(
                    sinks[p * HL : (p + 1) * HL].reshape(1, HL), (TILE, HL)
                ).astype(np.float32),
            ],
            axis=1,
        )
        in_maps.append({"q": qi[tok, hsl], "kv": kvi[tok, kvsl], "aux": aux})
        shards.append((tok, hsl))

    res = run_bass_kernel_spmd(nc, in_maps, core_ids=list(range(8)))
    o_list = [res.results[c]["o"] for c in range(8)]
    with jax.default_device(jits["cpu"]):
        out = np.asarray(jits["assemble"](o_list, B, S, PB, HL))
    return out


# revision 7
# speedup vs baseline: 1.0466x; 1.0466x over previous
"""Varlen causal sliding-window attention with per-head sink logits, on 8 trn2 cores.

Wall time under the axon tunnel is transfer-bound (~20 ms/MiB each way), so the
wire format is int8 with per-(token,head) scales for q/k/v and int8 with
per-output-channel scales for O; all dequant/requant runs on-device where
compute is ~free (cost-model exec ~0.3 ms vs seconds of transfer).

Sharding: data-parallel over (batch, head-group). Each core gets one batch's
tokens and 16/PB contiguous q-heads (PB = 8//B parts per batch) plus the
matching kv-heads.

Device inputs (per core):
  q [S, HL*128] int8, qs [128, HL*NT] f32   (qs[p, h*NT+t] = scale of token
  t*128+p, head h); k/v [S, KVL*128] int8 with sks/vs [128, KVL*NT] f32
  (sks premultiplied by softmax SCALE); sinks [128, HL] f32.
Device output:
  o [HL*128, S+4] int8 (O^T layout); the last 4 bytes of each row are the
  bitcast f32 per-channel absmax m (host dequant is o[:, :S] * m/127).

Device kernel per head:
  - load: int8 tiles DMA'd natural-layout; q dequants per-token via DVE
    tensor_scalar (int8 x [128,1] f32 -> f16) then PE-transposes per 128-block
    into qT; k converts int8->f16 (scales NOT applied) and PE-transposes into
    kT; v dequants in place (lhsT consumed natural).
  - QK^T per 128-key tile into PSUM; exp evicts PSUM->SBUF f16 probs with the
    per-key scale folded into the activation's per-partition scale AP
    (logit = qdeq . k_int, exp scale = SCALE*ks_key); triangular masks fix the
    two band edges.
  - PV + ones-matmul denominator per 256-query span accumulate O^T and D in
    one PSUM bank; DVE adds exp(sink), reciprocal, multiply into an f32 O^T
    row; after all spans: rowmax(|O|), quantize row to int8, DMA out.
"""

import sys

sys.path.insert(0, "/opt/trn_rl_repo")

import numpy as np

NUM_HEADS = 16
NUM_KV_HEADS = 4
HEAD_DIM = 128
WINDOW = 1024
SCALE = 0.08838834764831845
TILE = 128

_CACHE = {}
_JIT = {}


def _band_width(kj, S):
    # keys in tile kj are visible to queries q with 0 <= q - k <= WINDOW
    # -> q in [kj*TILE, kj*TILE + WINDOW + TILE), clipped to S
    return min(S, kj * TILE + WINDOW + TILE) - kj * TILE


def _chunks(w):
    # split [0, w) at 512 boundaries (PSUM bank) for matmul outputs
    out = []
    c0 = 0
    while c0 < w:
        out.append((c0, min(512, w - c0)))
        c0 += 512
    return out


def build_nc(S, HL, KVL):
    import concourse.bacc as bacc
    import concourse.mybir as mybir
    from concourse.masks import make_identity, make_lower_triangular, make_upper_triangular
    from concourse.tile import TileContext

    f32 = mybir.dt.float32
    f16 = mybir.dt.float16
    i8 = mybir.dt.int8
    NT = S // TILE
    WMAX = min(S, WINDOW + TILE)
    SUMW = sum(_band_width(kj, S) for kj in range(NT))
    OFF = np.cumsum([0] + [_band_width(kj, S) for kj in range(NT)]).tolist()
    SPAN = 256
    NSPAN = S // SPAN

    nc = bacc.Bacc()
    q_d = nc.dram_tensor("q", [S, HL * TILE], i8, kind="ExternalInput")
    # kv: per local kv head, 128 k-cols then 128 v-cols (interleaved per head)
    kv_d = nc.dram_tensor("kv", [S, KVL * 2 * TILE], i8, kind="ExternalInput")
    # aux columns: qs [HL*NT] | sks [KVL*NT] | vs [KVL*NT] | sinks [HL]
    aux_d = nc.dram_tensor(
        "aux", [TILE, HL * NT + 2 * KVL * NT + HL], f32, kind="ExternalInput"
    )
    # last 4 int8 cols of each row hold the f32 per-channel absmax (bitcast)
    o_d = nc.dram_tensor("o", [HL * TILE, S + 4], i8, kind="ExternalOutput")

    with TileContext(nc) as tc:
        with (
            tc.tile_pool(name="const", bufs=1) as const_pool,
            tc.tile_pool(name="qi8", bufs=2) as qi8_pool,
            tc.tile_pool(name="kvi8", bufs=2) as kvi8_pool,
            tc.tile_pool(name="dq", bufs=3) as dq_pool,
            tc.tile_pool(name="qT", bufs=3) as qT_pool,
            tc.tile_pool(name="kT", bufs=2) as kT_pool,
            tc.tile_pool(name="vv", bufs=2) as v_pool,
            tc.tile_pool(name="pT", bufs=3) as pT_pool,
            tc.tile_pool(name="dsb", bufs=3) as d_pool,
            tc.tile_pool(name="orow", bufs=2) as orow_pool,
            tc.tile_pool(name="oi8", bufs=2) as oi8_pool,
            tc.tile_pool(name="stat", bufs=4) as stat_pool,
            tc.tile_pool(name="spsum", bufs=1, space="PSUM") as s_psum,
            tc.tile_pool(name="opsum", bufs=2, space="PSUM") as o_psum,
            tc.tile_pool(name="tpsum", bufs=2, space="PSUM") as t_psum,
        ):
            mask_diag = const_pool.tile([TILE, TILE], f16)  # valid: q >= k
            mask_win = const_pool.tile([TILE, TILE], f16)  # valid: q <= k
            make_upper_triangular(nc, mask_diag[:], val=1.0, diag=True)
            make_lower_triangular(nc, mask_win[:], val=1.0, diag=True)
            ones = const_pool.tile([TILE, TILE], f16)
            nc.vector.memset(ones[:], 1.0)
            ident = const_pool.tile([TILE, TILE], f16)
            make_identity(nc, ident[:])
            AUXW = HL * NT + 2 * KVL * NT + HL
            aux_sb = const_pool.tile([TILE, AUXW], f32)
            nc.sync.dma_start(out=aux_sb[:], in_=aux_d[:, :])
            esk = const_pool.tile([TILE, HL], f32)
            nc.scalar.activation(
                esk[:],
                aux_sb[:, HL * NT + 2 * KVL * NT :],
                mybir.ActivationFunctionType.Exp,
            )

            kT_sb = None
            v_by_kv = {}
            pT_by_hl = {}

            def qk_phase(hl):
                nonlocal kT_sb
                kv = hl // 4 if HL >= 4 else 0
                if hl % 4 == 0 or kT_sb is None:
                    # ---- K: int8 natural -> f16 (unscaled) -> kT via PE ----
                    ki8 = kvi8_pool.tile([TILE, NT * TILE], i8, tag="ki8")
                    nc.sync.dma_start(
                        out=ki8[:].rearrange("p (t d) -> p t d", d=TILE),
                        in_=kv_d[:, kv * 2 * TILE : kv * 2 * TILE + TILE].rearrange(
                            "(t p) d -> p t d", p=TILE
                        ),
                    )
                    kT_sb = kT_pool.tile([TILE, S], f16, tag="kT")
                    for t in range(NT):
                        dq = dq_pool.tile([TILE, TILE], f16, tag="dq")
                        nc.vector.tensor_copy(dq[:], ki8[:, t * TILE : (t + 1) * TILE])
                        tp = t_psum.tile([TILE, TILE], f16, tag="tp")
                        nc.tensor.transpose(tp[:], dq[:], ident[:])
                        nc.vector.tensor_copy(kT_sb[:, t * TILE : (t + 1) * TILE], tp[:])
                    # ---- V: int8 natural, dequant in place ----
                    vi8 = kvi8_pool.tile([TILE, NT * TILE], i8, tag="vi8")
                    nc.gpsimd.dma_start(
                        out=vi8[:].rearrange("p (t d) -> p t d", d=TILE),
                        in_=kv_d[
                            :, kv * 2 * TILE + TILE : (kv + 1) * 2 * TILE
                        ].rearrange("(t p) d -> p t d", p=TILE),
                    )
                    v_sb = v_pool.tile([TILE, NT * TILE], f16, tag="vv")
                    for t in range(NT):
                        nc.vector.tensor_scalar_mul(
                            v_sb[:, t * TILE : (t + 1) * TILE],
                            vi8[:, t * TILE : (t + 1) * TILE],
                            aux_sb[:, HL * NT + KVL * NT + kv * NT + t : HL * NT + KVL * NT + kv * NT + t + 1],
                        )
                    v_by_kv[kv] = v_sb

                # ---- Q: int8 natural -> dequant (per-token scale) -> qT ----
                qi8 = qi8_pool.tile([TILE, NT * TILE], i8, tag="qi8")
                nc.sync.dma_start(
                    out=qi8[:].rearrange("p (t d) -> p t d", d=TILE),
                    in_=q_d[:, hl * TILE : (hl + 1) * TILE].rearrange(
                        "(t p) d -> p t d", p=TILE
                    ),
                )
                qT_sb = qT_pool.tile([TILE, S], f16, tag="qT")
                for t in range(NT):
                    dq = dq_pool.tile([TILE, TILE], f16, tag="dq")
                    nc.vector.tensor_scalar_mul(
                        dq[:],
                        qi8[:, t * TILE : (t + 1) * TILE],
                        aux_sb[:, hl * NT + t : hl * NT + t + 1],
                    )
                    tp = t_psum.tile([TILE, TILE], f16, tag="tp")
                    nc.tensor.transpose(tp[:], dq[:], ident[:])
                    nc.vector.tensor_copy(qT_sb[:, t * TILE : (t + 1) * TILE], tp[:])

                pT = pT_pool.tile([TILE, SUMW], f16, tag="pT")
                pT_by_hl[hl] = pT

                # ---- QK^T + exp (per-key scale via activation scale AP) ----
                for kj in range(NT):
                    w = _band_width(kj, S)
                    off = OFF[kj]
                    q0 = kj * TILE
                    s_ps = s_psum.tile([TILE, WMAX], f32, tag="s")
                    for c0, cw in _chunks(w):
                        nc.tensor.matmul(
                            s_ps[:, c0 : c0 + cw],
                            lhsT=kT_sb[:, kj * TILE : (kj + 1) * TILE],
                            rhs=qT_sb[:, q0 + c0 : q0 + c0 + cw],
                            start=True,
                            stop=True,
                        )
                    nc.scalar.activation(
                        pT[:, off : off + w],
                        s_ps[:, :w],
                        mybir.ActivationFunctionType.Exp,
                        scale=aux_sb[:, HL * NT + kv * NT + kj : HL * NT + kv * NT + kj + 1],
                    )
                    nc.vector.tensor_mul(
                        pT[:, off : off + TILE],
                        pT[:, off : off + TILE],
                        mask_diag[:],
                    )
                    if kj * TILE + WINDOW + TILE <= S:
                        nc.vector.tensor_mul(
                            pT[:, off + WINDOW : off + WINDOW + TILE],
                            pT[:, off + WINDOW : off + WINDOW + TILE],
                            mask_win[:],
                        )

            def pv_phase(hl):
                kv = hl // 4 if HL >= 4 else 0
                v_sb = v_by_kv[kv]
                pT = pT_by_hl.pop(hl)
                out_row = orow_pool.tile([TILE, S], f32, tag="orow")
                # ---- PV + denominator, per query span ----
                # od_ps: one PSUM bank; cols [0,SPAN) = O^T, [SPAN,2*SPAN) = D
                for sp in range(NSPAN):
                    lo, hi = sp * SPAN, (sp + 1) * SPAN
                    ktiles = []
                    for kj in range(NT):
                        w = _band_width(kj, S)
                        qlo = max(kj * TILE, lo)
                        qhi = min(kj * TILE + w, hi)
                        if qhi > qlo:
                            ktiles.append((kj, qlo, qhi))
                    # full-span writers first (uniform psum zero-region state)
                    ktiles.sort(key=lambda t: 0 if (t[1] == lo and t[2] == hi) else 1)
                    assert ktiles[0][1] == lo and ktiles[0][2] == hi, (S, sp)

                    od_ps = o_psum.tile([TILE, 2 * SPAN], f32, tag="od")
                    n = len(ktiles)
                    for i, (kj, qlo, qhi) in enumerate(ktiles):
                        rel_p = OFF[kj] + (qlo - kj * TILE)
                        rel_o = qlo - lo
                        ln = qhi - qlo
                        rhs = pT[:, rel_p : rel_p + ln]
                        nc.tensor.matmul(
                            od_ps[:, rel_o : rel_o + ln],
                            lhsT=v_sb[:, kj * TILE : (kj + 1) * TILE],
                            rhs=rhs,
                            start=(i == 0),
                            stop=False,
                        )
                        nc.tensor.matmul(
                            od_ps[:, SPAN + rel_o : SPAN + rel_o + ln],
                            lhsT=ones[:, :],
                            rhs=rhs,
                            start=False,
                            stop=(i == n - 1),
                        )

                    d_sb = d_pool.tile([TILE, SPAN], f32, tag="d_sb")
                    nc.vector.tensor_scalar_add(
                        d_sb[:], od_ps[:, SPAN : 2 * SPAN], esk[:, hl : hl + 1]
                    )
                    nc.vector.reciprocal(d_sb[:], d_sb[:])
                    nc.vector.tensor_mul(out_row[:, lo:hi], od_ps[:, :SPAN], d_sb[:])

                # ---- per-channel int8 quantization of the O^T row ----
                m = stat_pool.tile([TILE, 1], f32, tag="m")
                nc.vector.tensor_reduce(
                    out=m[:],
                    in_=out_row[:],
                    axis=mybir.AxisListType.X,
                    op=mybir.AluOpType.max,
                    apply_absolute_value=True,
                )
                nc.vector.tensor_scalar_max(m[:], m[:], 1e-20)
                r = stat_pool.tile([TILE, 1], f32, tag="r")
                nc.vector.reciprocal(r[:], m[:])
                nc.vector.tensor_scalar_mul(r[:], r[:], 127.0)
                oi8 = oi8_pool.tile([TILE, S], i8, tag="oi8")
                nc.vector.tensor_scalar_mul(oi8[:], out_row[:], r[:, 0:1])
                # out-DMA on SWDGE: keeps SP's FIFO free for the next
                # head's loads
                nc.gpsimd.dma_start(
                    out=o_d[hl * TILE : (hl + 1) * TILE, :S], in_=oi8[:]
                )
                nc.sync.dma_start(
                    out=o_d[hl * TILE : (hl + 1) * TILE, S : S + 4],
                    in_=m[:].bitcast(i8),
                )

            # software pipeline across heads: QK(hl+1) is emitted before
            # PV(hl) so PV never chases a just-issued exp
            qk_phase(0)
            for hl in range(1, HL):
                qk_phase(hl)
                pv_phase(hl - 1)
            pv_phase(HL - 1)
    # Bacc lowering (wait splitting, reg alloc) must run before serialization;
    # nothing on the PJRT path calls it for us.
    nc.finalize()
    return nc


def _get_nc(S, HL, KVL):
    key = (S, HL, KVL)
    if key not in _CACHE:
        _CACHE[key] = build_nc(S, HL, KVL)
    return _CACHE[key]


def _get_jits():
    if "prep" in _JIT:
        return _JIT
    import jax
    import jax.numpy as jnp

    _JIT["cpu"] = jax.local_devices(backend="cpu")[0]

    def quant(x, nh):
        T = x.shape[0]
        xr = x.reshape(T, nh, HEAD_DIM)
        s = jnp.maximum(jnp.max(jnp.abs(xr), axis=2) / 127.0, 1e-12)  # [T,nh]
        xi = jnp.clip(jnp.round(xr / s[:, :, None]), -127, 127).astype(jnp.int8)
        return xi.reshape(T, nh * HEAD_DIM), s

    def prep(q, k, v):
        qi, qs = quant(q, NUM_HEADS)
        ki, ks = quant(k, NUM_KV_HEADS)
        vi, vs = quant(v, NUM_KV_HEADS)
        T = k.shape[0]
        kvi = jnp.concatenate(
            [
                ki.reshape(T, NUM_KV_HEADS, HEAD_DIM),
                vi.reshape(T, NUM_KV_HEADS, HEAD_DIM),
            ],
            axis=2,
        ).reshape(T, NUM_KV_HEADS * 2 * HEAD_DIM)
        return qi, qs, kvi, ks, vs

    def assemble(o_list, B, S, PB, HL):
        raw = jnp.stack(o_list)  # [8, HL*128, S+4] int8
        oi8 = raw[:, :, :S]
        om = jax.lax.bitcast_convert_type(raw[:, :, S : S + 4], jnp.float32)
        om = om[:, :, None]  # [8, HL*128, 1] f32
        o = oi8.astype(jnp.float32) * (om / 127.0)  # [8, HL*128, S]
        o = o.reshape(B, PB, HL * HEAD_DIM, S)
        # out[b*S + s_, p*HL*128 + c] = o[b, p, c, s_]
        out = o.transpose(0, 3, 1, 2).reshape(B * S, NUM_HEADS * HEAD_DIM)
        return out

    _JIT["prep"] = jax.jit(prep)
    _JIT["assemble"] = jax.jit(assemble, static_argnums=(1, 2, 3, 4))
    return _JIT


def _scales_tile(s, scale_mul=1.0):
    """[S, nh] f32 per-core scales -> [128, nh*NT] device layout
    (column h*NT+t holds scales of tokens t*128 + p)."""
    S, nh = s.shape
    NT = S // TILE
    out = (s.reshape(NT, TILE, nh) * scale_mul).transpose(1, 2, 0)
    return np.ascontiguousarray(out.reshape(TILE, nh * NT), dtype=np.float32)


def kernel(q, k, v, sinks, batch, seqlen):
    import jax

    from concourse.bass_utils import run_bass_kernel_spmd

    q = np.asarray(q)
    k = np.asarray(k)
    v = np.asarray(v)
    sinks = np.asarray(sinks)
    B = int(batch)
    S = int(seqlen)
    assert 8 % B == 0, B
    PB = 8 // B  # head-parts per batch
    HL = NUM_HEADS // PB
    KVL = max(1, NUM_KV_HEADS // PB)

    nc = _get_nc(S, HL, KVL)
    jits = _get_jits()

    with jax.default_device(jits["cpu"]):
        qi, qs, kvi, ks, vs = [np.asarray(a) for a in jits["prep"](q, k, v)]

    in_maps = []
    for c in range(8):
        b, p = divmod(c, PB)
        tok = slice(b * S, (b + 1) * S)
        hsl = slice(p * HL * HEAD_DIM, (p + 1) * HL * HEAD_DIM)
        kv_lo = (p * HL) // 4
        kvsl = slice(kv_lo * 2 * HEAD_DIM, (kv_lo + KVL) * 2 * HEAD_DIM)
        aux = np.concatenate(
            [
                _scales_tile(qs[tok, p * HL : (p + 1) * HL]),
                _scales_tile(ks[tok, kv_lo : kv_lo + KVL], SCALE),
                _scales_tile(vs[tok, kv_lo : kv_lo + KVL]),
                np.broadcast_to(
                    sinks[p * HL : (p + 1) * HL].reshape(1, HL), (TILE, HL)
                ).astype(np.float32),
            ],
            axis=1,
        )
        in_maps.append({"q": qi[tok, hsl], "kv": kvi[tok, kvsl], "aux": aux})

    res = run_bass_kernel_spmd(nc, in_maps, core_ids=list(range(8)))
    o_list = [res.results[c]["o"] for c in range(8)]
    with jax.default_device(jits["cpu"]):
        out = np.asarray(jits["assemble"](o_list, B, S, PB, HL))
    return out
